# revision 103
# baseline (speedup 1.0000x reference)
"""Causal multi-head self-attention block (B=2, T=2048, C=1024, H=16) on 8
Trainium2 NeuronCores.

Sharding: core c = 4*b + g handles batch b (2-way data parallel) and head
group g (4-way tensor parallel over the 16 heads -> 4 heads/core).
c_attn is column-sharded (each core computes K/Q/V features only for its 4
heads); c_proj is row-sharded (each core contracts its 4 heads' attn output
against the matching w_proj columns and emits a full-width partial output).
The 4 partial outputs per batch are summed on the host (+ b_proj).

Per-core device pipeline (all matmuls bf16 with fp32 PSUM accumulation):
  1. KQ^T = (w_kq x)        -> [feat, T] layout, feat on partitions
  2. V    = (x^T w_v^T)     -> [T, d] natural layout, augmented with a
     ones column so the AV matmul also yields the softmax denominators
  3. per head pair, per 512-wide q chunk, over live (causal) k tiles:
       aff^T[k,q] for both heads -> one [128, 2, 512] PSUM pair (2 banks)
       E = exp(0.125*aff^T)      -> ONE wide ScalarE activation per tile
                                    (strided view on diagonal tiles), bf16
       diagonal-tile mask multiply runs on the Pool engine
       [attn^T unnorm; sums] += V_aug.T E   (M=65, per head)
     epilogue: reciprocal rows (DVE), partition_broadcast (Pool),
     normalize (DVE).  No PE involvement in the epilogue.
  4. out_partial = attn^T.T w_proj_slice -> PSUM, DMA'd straight to HBM.

Because each engine executes its compiled instruction stream strictly in
order, emission order is scheduling: aff runs many tiles ahead of AV
(hides the ScalarE exp latency and any transient backlog), block
epilogues are deferred into the next block, and filler matmuls (K/Q for
the other head pair, V tiles, output projection) are injected mid-block
wherever ScalarE would otherwise be the per-tile rate limiter.  Startup
DMAs are ordered critical-first (wkq's first half is packed with xT's
first chunk in one host tensor), and the final block normalizes its
output 128 columns at a time — pav columns complete in a staircase
because later diagonal tiles never touch earlier columns — so the last
projections overlap the last attention tiles.
"""

import os
import sys

for _p in ("/opt/trn_rl_repo",):
    if os.path.isdir(_p) and _p not in sys.path:
        sys.path.append(_p)

import numpy as np
import ml_dtypes

B, T, C, H, D = 2, 2048, 1024, 16, 64
N_CORES = 8
HPC = H // 4          # heads per core = 4
CPC = HPC * D         # attn feature cols per core = 256
KQF = 2 * CPC         # K+Q features per core = 512
TCH = 512             # q-chunk width
NJ = T // TCH         # 4 q chunks
NTI = T // 128        # 16 t tiles

_CACHE = {}


def _build_program():
    from contextlib import ExitStack

    import concourse.bass as bass
    import concourse.mybir as mybir
    import concourse.tile as tile
    from concourse import bacc
    from concourse.bass import ts

    f32 = mybir.dt.float32
    bf16 = mybir.dt.bfloat16
    Exp = mybir.ActivationFunctionType.Exp

    nc = bacc.Bacc("TRN2", target_bir_lowering=False, debug=False,
                   num_devices=N_CORES)

    xq0_d = nc.dram_tensor("xq0", [128, 8, 256 + TCH], bf16,
                           kind="ExternalInput")
    xT_d = nc.dram_tensor("xT", [128, 8, T], bf16, kind="ExternalInput")
    wkq_d = nc.dram_tensor("wkq", [128, 8, KQF], bf16, kind="ExternalInput")
    bkq_d = nc.dram_tensor("bkq", [128, 4], f32, kind="ExternalInput")
    wv_d = nc.dram_tensor("wv", [128, 8, CPC], bf16, kind="ExternalInput")
    wp_d = nc.dram_tensor("wp", [128, 2, C], bf16, kind="ExternalInput")
    mask_d = nc.dram_tensor("mask", [128, 2, 128], bf16, kind="ExternalInput")
    out_d = nc.dram_tensor("out", [T, C], bf16, kind="ExternalOutput")

    with tile.TileContext(nc) as tc, ExitStack() as ctx:
        pp = ctx.enter_context(tc.tile_pool(name="persist", bufs=1))
        xq0_sb = pp.tile([128, 8, 256 + TCH], bf16)
        xT_sb = pp.tile([128, 8, T], bf16)
        wkq_sb = pp.tile([128, 8, KQF], bf16)
        bkq_sb = pp.tile([128, 4], f32)
        wv_sb = pp.tile([128, 8, CPC], bf16)
        wp_sb = pp.tile([128, 2, C], bf16)
        mask_sb = pp.tile([128, 2, 128], bf16)
        kq_sb = pp.tile([128, 4, T], bf16)
        v_sb = pp.tile([128, NTI, HPC, D + 1], bf16)
        attn_sb = pp.tile([128, 2, T], bf16)

        # critical path first.  Host orders wkq features [K01, Q01, K23,
        # Q23], so the half the first head-pair needs is one strided DMA;
        # xT's first chunk goes per-c so KQ matmuls unlock progressively.
        # weights ride the Pool engine's SWDGE queue so their descriptor
        # generation runs in parallel with the xT stream's on HWDGE
        nc.sync.dma_start(xq0_sb[:, 0, :], xq0_d[:, 0, :])
        nc.gpsimd.dma_start(bkq_sb[:], bkq_d[:])
        nc.sync.dma_start(xq0_sb[:, 1:3, :], xq0_d[:, 1:3, :])
        nc.sync.dma_start(xq0_sb[:, 3:5, :], xq0_d[:, 3:5, :])
        nc.sync.dma_start(xq0_sb[:, 5:6, :], xq0_d[:, 5:6, :])
        nc.sync.dma_start(xq0_sb[:, 6:7, :], xq0_d[:, 6:7, :])
        nc.sync.dma_start(xq0_sb[:, 7:8, :], xq0_d[:, 7:8, :])
        nc.sync.dma_start(wv_sb[:], wv_d[:])
        nc.sync.dma_start(wkq_sb[:, :, 256:512], wkq_d[:, :, 256:512])
        for tch in range(1, NJ):
            nc.sync.dma_start(xT_sb[:, :, ts(tch, TCH)],
                              xT_d[:, :, ts(tch, TCH)])
        nc.sync.dma_start(wp_sb[:], wp_d[:])
        nc.gpsimd.dma_start(mask_sb[:], mask_d[:])
        for ti in range(NTI):
            nc.any.memset(v_sb[:, ti, :, D:D + 1], 1.0)

        # PSUM: aff pairs 2x[128,2,512] (4 banks) + acc 2x[128,512]
        # (2 banks) + work 2x[128,512] (2 banks) = 8 banks.
        pa_pool = ctx.enter_context(
            tc.tile_pool(name="pall", bufs=1, space="PSUM"))
        e_pool = ctx.enter_context(tc.tile_pool(name="epool", bufs=1))
        r_pool = ctx.enter_context(tc.tile_pool(name="rpool", bufs=1))
        o_pool = ctx.enter_context(tc.tile_pool(name="outp", bufs=1))

        def wkq_at(c, m):
            # feature tiles m0/m1 arrive packed with xT chunk 0 in xq0
            if m < 2:
                return xq0_sb[:, c, ts(m, 128)]
            return wkq_sb[:, c, ts(m, 128)]

        def xT_at(c, t0, t1):
            if t1 <= TCH:
                return xq0_sb[:, c, 256 + t0:256 + t1]
            return xT_sb[:, c, t0:t1]

        def emit_kq_tiles(ms, tch):
            # interleave the c-loops of several feature tiles so the PE can
            # advance as each 128-row chunk of x arrives from HBM
            pk = {m: pa_pool.tile([128, TCH], f32, tag="work", bufs=2,
                                  name="pkq") for m in ms}
            for c in range(8):
                for m in ms:
                    nc.tensor.matmul(
                        pk[m][:], wkq_at(c, m),
                        xT_at(c, tch * TCH, (tch + 1) * TCH),
                        start=(c == 0), stop=(c == 7))
            for m in ms:
                nc.vector.tensor_scalar_add(
                    kq_sb[:, m, ts(tch, TCH)], pk[m][:], bkq_sb[:, m:m + 1])

        def emit_kq_tile(m, tch):
            emit_kq_tiles([m], tch)

        def emit_v(ti):
            pv = pa_pool.tile([128, CPC], f32, tag="work", bufs=2, name="pv")
            for c in range(8):
                nc.tensor.matmul(
                    pv[:], xT_at(c, ti * 128, (ti + 1) * 128), wv_sb[:, c, :],
                    start=(c == 0), stop=(c == 7))
            nc.vector.tensor_copy(
                v_sb[:, ti, :, 0:D],
                pv[:].rearrange("p (h d) -> p h d", h=HPC))

        def emit_attn_block(g, j, hooks=(), fillers=(), filler_start=2,
                            tail_units=None):
            """Emit one (head-pair, q-chunk) attention block.

            `hooks` run once at tile 1 (used for the previous block's
            deferred epilogue).  `fillers` are closures emitting ~1-2 PE
            matmuls each; they are drained one per tile from tile 4 on, so
            the PE has independent work wherever ScalarE exp would
            otherwise gate the AV matmuls.  Returns the epilogue closure.
            """
            pav0 = pa_pool.tile([128, TCH], f32, tag="acc", bufs=2,
                                name="pav0")
            pav1 = pa_pool.tile([128, TCH], f32, tag="acc", bufs=2,
                                name="pav1")
            n_live = 4 * j + 4
            es = {}
            fillq = list(fillers)

            def emit_aff(i):
                # diagonal tiles only touch queries q >= k: narrow the
                # q-range to [q0:TCH]
                q0 = max(0, 128 * i - TCH * j)
                qsl = slice(j * TCH + q0, (j + 1) * TCH)
                ap = pa_pool.tile([128, 2, TCH], f32, tag="aff", bufs=2,
                                  name="affp")
                nc.tensor.matmul(
                    ap[:, 0, q0:], kq_sb[0:64, 2 * g, ts(i, 128)],
                    kq_sb[0:64, 2 * g + 1, qsl], start=True, stop=True)
                nc.tensor.matmul(
                    ap[:, 1, q0:], kq_sb[64:128, 2 * g, ts(i, 128)],
                    kq_sb[64:128, 2 * g + 1, qsl], start=True, stop=True)
                ep = e_pool.tile([128, 2, TCH], bf16, tag="e", bufs=16,
                                 name="ep")
                nc.scalar.activation(ep[:, :, q0:], ap[:, :, q0:], Exp,
                                     scale=0.125)
                if q0 > 0 or i == 4 * j:
                    nc.vector.tensor_mul(
                        ep[:, :, q0:q0 + 128], ep[:, :, q0:q0 + 128],
                        mask_sb[:])
                es[i] = (ep, q0)

            def emit_av(i):
                ep, q0 = es.pop(i)
                first, last = (i == 0), (i == n_live - 1)
                nc.tensor.matmul(
                    pav0[0:65, q0:], v_sb[:, i, 2 * g + 0, :],
                    ep[:, 0, q0:], start=first, stop=last)
                nc.tensor.matmul(
                    pav1[0:65, q0:], v_sb[:, i, 2 * g + 1, :],
                    ep[:, 1, q0:], start=first, stop=last)

            def finalize_cols(sl):
                r2 = r_pool.tile([1, 2, TCH], f32, tag="r2", bufs=2)
                nc.vector.reciprocal(r2[0:1, 0, sl], pav0[64:65, sl])
                nc.vector.reciprocal(r2[0:1, 1, sl], pav1[64:65, sl])
                rb2 = r_pool.tile([128, 2, TCH], f32, tag="rb2", bufs=2)
                nc.gpsimd.partition_broadcast(rb2[:, :, sl], r2[0:1, :, sl],
                                              channels=128)
                qsl = slice(j * TCH + sl.start, j * TCH + sl.stop)
                nc.vector.tensor_mul(
                    attn_sb[0:64, g, qsl], pav0[0:64, sl], rb2[0:64, 0, sl])
                nc.vector.tensor_mul(
                    attn_sb[64:128, g, qsl], pav1[0:64, sl],
                    rb2[64:128, 1, sl])

            look = min(14, n_live)
            for i in range(n_live):
                emit_aff(i)
                if i == 1:
                    for h in hooks:
                        h()
                if i >= look:
                    emit_av(i - look)
                if fillq and i >= filler_start and i % 2 == 1:
                    fillq.pop(0)()
            for i in range(n_live - look, n_live):
                emit_av(i)
                if tail_units is not None and i >= 4 * j:
                    # staircase completion: pav columns [qb*128:(qb+1)*128]
                    # are final right after av(4j+qb), so normalize each
                    # piece early.  Each projection's ct0-half matmul is
                    # emitted eagerly (it reads long-finished g0 attn); only
                    # the ct1-half + copy + DMA wait on the piece chain, one
                    # piece behind, so the PE never idles on the epilogue.
                    qb = i - 4 * j
                    if qb >= 1:
                        tail_units[2 * (qb - 1)].b()
                        tail_units[2 * qb - 1].b()
                    if fillq:
                        fillq.pop(0)()
                    finalize_cols(slice(qb * 128, (qb + 1) * 128))
                    tail_units[2 * qb].a()
                    tail_units[2 * qb + 1].a()
                elif fillq:
                    fillq.pop(0)()
            for f in fillq:
                f()
            if tail_units is not None:
                tail_units[6].b()
                tail_units[7].b()

            def finalize():
                finalize_cols(slice(0, TCH))

            finalize.cols = finalize_cols
            return finalize

        def proj_units(j, tail=False):
            # (ti, och) units; the two och halves share one [128,1024] SBUF
            # staging tile so each ti goes out as a single DMA.  At the
            # kernel tail each unit is split: `.a` emits the ct0-half
            # matmul eagerly, `.b` the piece-gated ct1-half + copy + DMA
            # (och0 copy on the by-then-idle ScalarE so DVE keeps pace).
            units = []
            for ti in range(4 * j, 4 * j + 4):
                ots = {}

                def u_a(ti=ti, och=0, ots=ots, pos={}):
                    if och == 0:
                        ots[0] = o_pool.tile([128, C], bf16, tag="ot",
                                             bufs=3, name="ot")
                    po = pa_pool.tile([128, 512], f32, tag="work", bufs=2,
                                      name="po")
                    pos[och] = po
                    nc.tensor.matmul(
                        po[:], attn_sb[:, 0, ts(ti, 128)],
                        wp_sb[:, 0, ts(och, 512)], start=True, stop=False)
                    return pos

                def u_b(ti=ti, och=0, ots=ots, pos=None):
                    po = pos[och]
                    nc.tensor.matmul(
                        po[:], attn_sb[:, 1, ts(ti, 128)],
                        wp_sb[:, 1, ts(och, 512)], start=False, stop=True)
                    if tail and och == 0:
                        nc.scalar.copy(ots[0][:, ts(och, 512)], po[:])
                    else:
                        nc.vector.tensor_copy(ots[0][:, ts(och, 512)], po[:])
                    if tail:
                        nc.sync.dma_start(
                            out_d[ts(ti, 128), ts(och, 512)],
                            ots[0][:, ts(och, 512)])
                    elif och == 1:
                        nc.sync.dma_start(out_d[ts(ti, 128), :], ots[0][:])

                for och in range(2):
                    pos = {}

                    def mk(ti=ti, och=och, ots=ots, pos=pos):
                        def whole():
                            u_a(ti, och, ots, pos)
                            u_b(ti, och, ots, pos)
                        whole.a = lambda: u_a(ti, och, ots, pos)
                        whole.b = lambda: u_b(ti, och, ots, pos)
                        return whole
                    units.append(mk())
            return units

        def kq_filler(m, tch):
            return lambda: emit_kq_tile(m, tch)

        def v_filler(ti):
            return lambda: emit_v(ti)

        # loop 1: KQ/V production interleaved with g0 attention blocks.
        # Feature-tile order is [K01, Q01, K23, Q23]: g0 needs m0/m1; g1's
        # m2/m3 chunks are deferred into loop 2 as just-in-time fillers
        # (only chunk 0 must exist before block (g1,0) starts).
        fin = None
        emit_kq_tiles([0, 1], 0)
        for tch in range(NJ):
            if tch == 0:
                for ti in range(4):
                    emit_v(ti)
            fillers = []
            if tch < NJ - 1:
                fillers.append(kq_filler(0, tch + 1))
                fillers.append(kq_filler(1, tch + 1))
                fillers.extend(v_filler(ti)
                               for ti in range(4 * tch + 4, 4 * tch + 8))
            else:
                fillers.append(kq_filler(2, 0))
                fillers.append(kq_filler(3, 0))
            hooks = [fin] if fin else []
            fin = emit_attn_block(0, tch, hooks=hooks, fillers=fillers,
                                  filler_start=2)

        # loop 2: g1 attention blocks ascending; each block's fillers are
        # the NEXT chunk's K23/Q23 tiles plus the projection units of the
        # previously finalized chunk.
        # order [0, 1, 3, 2]: the 12-tile chunk-2 block runs LAST because
        # the kernel's serial tail residue is the final block's ScalarE act
        # chain plus its epilogue — 4 tiles shorter here than chunk 3's.
        prev_j = None
        kq_next = {0: 1, 1: 3, 3: 2}
        carry = []
        for j in [0, 1, 3, 2]:
            hooks = [fin]
            fillers = []
            if j in kq_next:
                fillers.append(kq_filler(2, kq_next[j]))
                fillers.append(kq_filler(3, kq_next[j]))
            if prev_j is not None:
                units = proj_units(prev_j)
                if j == 3:
                    # rebalance: block (g1,3) is the longest PE span, the
                    # final block is chain-bound with PE slack — carry half
                    # of chunk 1's projection there (safe early fillers)
                    fillers.extend(units[:6])
                    carry = units[6:]
                else:
                    fillers.extend(units)
            if j == 2:
                fillers = carry + fillers
            tail_units = proj_units(2, tail=True) if j == 2 else None
            fin = emit_attn_block(1, j, hooks=hooks, fillers=fillers,
                                  filler_start=2 if j != 2 else 4,
                                  tail_units=tail_units)
            prev_j = j

    nc.compile()
    return nc


def _get_program():
    if "nc" not in _CACHE:
        _CACHE["nc"] = _build_program()
    return _CACHE["nc"]


def _host_mask():
    # mask[p, s, c] = 1.0 iff key-local p <= query-local c, duplicated over
    # s (the two heads of a pair share the mask)
    i = np.arange(128)[:, None]
    jj = np.arange(128)[None, :]
    m = (i <= jj).astype(ml_dtypes.bfloat16)
    return np.ascontiguousarray(
        np.broadcast_to(m[:, None, :], (128, 2, 128)))


def _shard_inputs(x, w_attn, b_attn, w_proj, b_proj):
    bf = ml_dtypes.bfloat16
    mask = _host_mask()
    in_maps = []
    for c in range(N_CORES):
        b, g = divmod(c, 4)
        hs = slice(g * CPC, (g + 1) * CPC)
        # xT: (C, T) -> (128, 8, T)
        xT = np.ascontiguousarray(
            x[b].T.reshape(8, 128, T).transpose(1, 0, 2)).astype(bf)
        # K block rows 0:C, Q rows C:2C, V rows 2C:3C of w_attn.  Feature
        # tiles ordered [K01, Q01, K23, Q23] so the first head pair's
        # K and Q are one contiguous half.
        Kr = w_attn[g * CPC:(g + 1) * CPC]
        Qr = w_attn[C + g * CPC:C + (g + 1) * CPC]
        wkq = np.concatenate([Kr[0:128], Qr[0:128],
                              Kr[128:256], Qr[128:256]], axis=0)
        # (KQF, C) -> transpose -> (C, KQF) -> (128, 8, KQF)
        wkq = np.ascontiguousarray(
            wkq.T.reshape(8, 128, KQF).transpose(1, 0, 2)).astype(bf)
        bK = b_attn[g * CPC:(g + 1) * CPC]
        bQ = b_attn[C + g * CPC:C + (g + 1) * CPC]
        bkq = np.concatenate([bK[0:128], bQ[0:128], bK[128:256],
                              bQ[128:256]])
        bkq = np.ascontiguousarray(bkq.reshape(4, 128).T).astype(np.float32)
        wv = w_attn[2 * C + g * CPC:2 * C + (g + 1) * CPC]  # (CPC, C)
        wv = np.ascontiguousarray(
            wv.T.reshape(8, 128, CPC).transpose(1, 0, 2)).astype(bf)
        wp = w_proj[:, hs].T  # (CPC, C)
        wp = np.ascontiguousarray(
            wp.reshape(2, 128, C).transpose(1, 0, 2)).astype(bf)
        xq0 = np.ascontiguousarray(
            np.concatenate([wkq[:, :, 0:256], xT[:, :, 0:TCH]], axis=2))
        in_maps.append({"xq0": xq0, "xT": xT, "wkq": wkq, "bkq": bkq,
                        "wv": wv, "wp": wp, "mask": mask})
    return in_maps


def kernel(x, w_attn, b_attn, w_proj, b_proj):
    from concourse.bass_utils import run_bass_kernel_spmd

    nc = _get_program()
    in_maps = _shard_inputs(x, w_attn, b_attn, w_proj, b_proj)
    res = run_bass_kernel_spmd(nc, in_maps, core_ids=list(range(N_CORES)))
    out = np.zeros((B, T, C), dtype=np.float32)
    for c in range(N_CORES):
        b = c // 4
        out[b] += res.results[c]["out"].astype(np.float32)
    # V-bias contribution folded out of the device kernel:
    # (attn + bv)^T @ wp  =  attn^T @ wp  +  (bv @ wp)
    bv_full = b_attn[2 * C:3 * C].astype(np.float64)
    bias_out = bv_full @ w_proj.T.astype(np.float64)
    out += (b_proj.astype(np.float64) + bias_out)[None, None, :].astype(
        np.float32)
    return out
